# revision 1
# baseline (speedup 1.0000x reference)
"""GCN encoder (2-layer GCNConv, PyG-default normalization) kernel.

Self-contained: takes FULL unsharded inputs, returns FULL output.

Hardcoded problem shape: N=50000 nodes, E=800000 edges, IN=128,
HID=128, OUT=64, f32 features / int32 edge indices.

Strategy
--------
The dominant cost is the edge gather + segment-sum (memory regime).
We implement segment_sum(msg, col) by sorting edges by target once
(np.argsort on int32 keys) and using np.add.reduceat over the sorted
message matrix, which is a single sequential pass over the 850k x D
message array — far faster than np.add.at scatter.  The dense
transforms (x @ W1, h @ W2) are small GEMMs done with BLAS.

A JAX-on-Neuron path is attempted first for the dense transforms when
trn2 devices are reachable; any failure falls back to the pure-host
path so the kernel always returns a correct result.
"""

import numpy as np

N_NODES = 50000
N_EDGES = 800000


def _segment_sum_sorted(msg, col_sorted_idx, starts, n_out, d):
    # msg already ordered by target via col_sorted_idx outside
    out = np.add.reduceat(msg, starts, axis=0)
    return out


def _gcn_host(x, edge_index, W1, b1, W2, b2):
    N = x.shape[0]
    row = edge_index[0].astype(np.int64)
    col = edge_index[1].astype(np.int64)
    loops = np.arange(N, dtype=np.int64)
    row_f = np.concatenate([row, loops])
    col_f = np.concatenate([col, loops])

    deg = np.bincount(col_f, minlength=N).astype(np.float32)
    dinv = np.where(deg > 0, 1.0 / np.sqrt(deg), 0.0).astype(np.float32)
    norm = (dinv[row_f] * dinv[col_f]).astype(np.float32)

    # Sort edges by target once; reuse for both layers.
    order = np.argsort(col_f, kind="stable")
    row_s = row_f[order]
    col_s = col_f[order]
    norm_s = norm[order][:, None]

    # Segment boundaries over the sorted targets. Every node has a
    # self-loop so every segment 0..N-1 is non-empty -> reduceat rows
    # map 1:1 onto nodes.
    starts = np.searchsorted(col_s, np.arange(N, dtype=np.int64))

    def conv(h, W, b):
        hw = h @ W
        msg = norm_s * hw[row_s]
        agg = np.add.reduceat(msg, starts, axis=0)
        return agg + b

    h = np.maximum(conv(x, W1, b1), 0.0)
    out = conv(h, W2, b2)
    return out.astype(np.float32)


def kernel(x, edge_index, W1, b1, W2, b2):
    x = np.asarray(x, dtype=np.float32)
    edge_index = np.asarray(edge_index)
    W1 = np.asarray(W1, dtype=np.float32)
    b1 = np.asarray(b1, dtype=np.float32)
    W2 = np.asarray(W2, dtype=np.float32)
    b2 = np.asarray(b2, dtype=np.float32)
    return _gcn_host(x, edge_index, W1, b1, W2, b2)



# revision 22
# speedup vs baseline: 8.1319x; 8.1319x over previous
"""GCN encoder (2-layer GCNConv, PyG-default normalization) on 8 trn2 NeuronCores.

Full inputs in, full output out. Internally:
  - nodes are padded to NPAD = 8*TPC and sharded contiguously across 8 cores
  - normalization is folded into the gather tables:
      table1 = (dinv*x) @ W1          (bf16, AllGathered to every core)
      conv1  = dinv * (segsum_edges(table1[row]) + table1[own]) + b1
      table2 = (dinv * relu(conv1)) @ W2   (bf16, padded to 128 cols, AllGathered)
      out    = dinv * (segsum_edges(table2[row]) + table2[own]) + b2
  - per-edge gathers run on-device via gpsimd dma_gather (int16 indices,
    lo/hi split around 32768); segment sums are TensorE matmuls against
    0/1 selection matrices built on VectorE from edge target values.

One SPMD program is prebuilt at import time with a data-independent uniform
chunk structure (per-tile lo/hi chunk counts sized for the random-graph tail)
and warmed up with a dummy run, so the real call only pays preprocessing +
upload + execute. If the actual data exceeds the uniform caps (or biases are
nonzero), a data-driven program is built on the fly instead.
"""

import math
import os

import numpy as np
import ml_dtypes

P = 128
C = 8  # cores
N_NODES = 50000
IN_CH = 128
HID = 128
OUT_CH = 64
LO_LIM = 32768

TPC = 6272  # targets per core; 49 tiles of 128
TILES = TPC // P
SEG_TILES = 7
# uniform per-tile chunk caps: lo edges/tile ~ Poisson(1337) sd 37 -> 13*128=1664
# hi ~ Poisson(704) sd 27 -> 7*128=896
KL_UNIFORM = 13
KH_UNIFORM = 7

_prog_cache = {}
_runner_cache = {}


# ---------------------------------------------------------------- host prep


def _chunk_layout(KL, KH, tiles, seg_tiles):
    nseg = math.ceil(tiles / seg_tiles)
    seg_tile_rng = [
        (s * seg_tiles, min((s + 1) * seg_tiles, tiles)) for s in range(nseg)
    ]
    lo_chunk_off = np.zeros(tiles, dtype=np.int64)
    hi_chunk_off = np.zeros(tiles, dtype=np.int64)
    seg_chunk_off = []
    k = 0
    for t0, t1 in seg_tile_rng:
        off = k
        for t in range(t0, t1):
            lo_chunk_off[t] = k
            k += KL[t]
        nlo_c = k - off
        for t in range(t0, t1):
            hi_chunk_off[t] = k
            k += KH[t]
        seg_chunk_off.append((off, int(nlo_c), int(k - off - nlo_c)))
    return nseg, seg_tile_rng, lo_chunk_off, hi_chunk_off, seg_chunk_off, int(k)


def _preprocess(x, edge_index, n_nodes, tpc, seg_tiles, lo_lim=LO_LIM, KL=None, KH=None, st_extra=None):
    """Returns (st, percore, fits). When KL/KH given (uniform caps), `fits`
    reports whether the data obeys them; otherwise caps are data-driven."""
    tiles = tpc // P
    npad = C * tpc

    row = np.ascontiguousarray(edge_index[0], dtype=np.int32)
    col = np.ascontiguousarray(edge_index[1], dtype=np.int32)

    deg = np.bincount(col, minlength=npad).astype(np.float32) + 1.0
    deg[n_nodes:] = 0.0
    dinv = np.zeros(npad, dtype=np.float32)
    dinv[:n_nodes] = 1.0 / np.sqrt(deg[:n_nodes])

    order = np.argsort(col, kind="stable")
    row_s = row[order]
    col_s = col[order]

    core_bounds = np.searchsorted(col_s, np.arange(C + 1, dtype=np.int32) * tpc)

    percore_edges = []
    n_lo = np.zeros((C, tiles), dtype=np.int64)
    n_hi = np.zeros((C, tiles), dtype=np.int64)
    for c in range(C):
        e0, e1 = core_bounds[c], core_bounds[c + 1]
        r = row_s[e0:e1]
        cl = col_s[e0:e1] - c * tpc
        tile_id = cl >> 7
        is_lo = r < lo_lim
        n_lo[c] = np.bincount(tile_id[is_lo], minlength=tiles)
        n_hi[c] = np.bincount(tile_id[~is_lo], minlength=tiles)
        percore_edges.append((r, cl, tile_id, is_lo))

    fits = True
    if KL is None:
        KL = np.maximum(1, np.ceil(n_lo.max(axis=0) / P).astype(np.int64)).tolist()
        KH = np.ceil(n_hi.max(axis=0) / P).astype(np.int64).tolist()
    else:
        KL = [KL] * tiles if np.isscalar(KL) else list(KL)
        KH = [KH] * tiles if np.isscalar(KH) else list(KH)
        if (n_lo.max(axis=0) > np.array(KL) * P).any() or (
            n_hi.max(axis=0) > np.array(KH) * P
        ).any():
            fits = False

    nseg, seg_tile_rng, lo_off, hi_off, seg_chunk_off, ncht = _chunk_layout(
        KL, KH, tiles, seg_tiles
    )

    st_pre = dict(tiles=tiles, ncht=ncht)
    wb1 = bool(st_extra.get("wb1")) if st_extra else False
    wb2 = bool(st_extra.get("wb2")) if st_extra else False
    offs, blob_w = _blob_fields(st_pre, wb1, wb2)
    blob = np.empty((C, blob_w), dtype=np.int16)
    if fits:
        lo_off32 = lo_off.astype(np.int32)
        hi_off32 = hi_off.astype(np.int32)
        for c in range(C):
            r, cl, tile_id, is_lo = percore_edges[c]
            # rank of each edge within its (tile, lo/hi) group
            tstart = np.searchsorted(cl, np.arange(tiles, dtype=np.int32) * P).astype(
                np.int32
            )
            pos_in_tile = np.arange(len(cl), dtype=np.int32) - tstart[tile_id]
            lo_cum = np.cumsum(is_lo).astype(np.int32)
            lo_before = np.concatenate([np.zeros(1, np.int32), lo_cum])[tstart]
            n_lo_upto = lo_cum - lo_before[tile_id]  # incl current if lo
            slot = np.where(
                is_lo,
                lo_off32[tile_id] * P + n_lo_upto - 1,
                hi_off32[tile_id] * P + pos_in_tile - n_lo_upto,
            )
            cv = np.full(ncht * P, -1, dtype=np.int16)
            ix = np.zeros(ncht * P, dtype=np.int16)
            cv[slot] = (cl & 127).astype(np.int16)
            ix[slot] = (np.where(is_lo, r, r - lo_lim)).astype(np.int16)
            o, rr, cc = offs["colv"]
            blob[c, o : o + rr * cc] = cv.reshape(ncht, P).T.reshape(-1)
            o, rr, cc = offs["idx"]
            blob[c, o : o + rr * cc] = ix.reshape(ncht * 8, 16).T.reshape(-1)

    dinv_sb = dinv.reshape(C, tiles, P).transpose(0, 2, 1)
    o, rr, cc = offs["dinv"]
    blob[:, o : o + rr * cc] = (
        np.ascontiguousarray(dinv_sb).view(np.int16).reshape(C, -1)
    )
    iota = np.tile(np.arange(P, dtype=np.float32), (P, 1)).astype(ml_dtypes.bfloat16)
    o, rr, cc = offs["iota"]
    blob[:, o : o + rr * cc] = iota.view(np.int16).reshape(-1)[None, :]

    st = dict(
        lo_lim=lo_lim,
        tiles=tiles,
        tpc=tpc,
        npad=npad,
        nseg=nseg,
        seg_tile_rng=seg_tile_rng,
        seg_chunk_off=seg_chunk_off,
        KL=list(KL),
        KH=list(KH),
        lo_chunk_off=lo_off.tolist(),
        hi_chunk_off=hi_off.tolist(),
        ncht=ncht,
    )
    return st, blob, fits


def _uniform_st(tpc=TPC, seg_tiles=SEG_TILES):
    tiles = tpc // P
    KL = [KL_UNIFORM] * tiles
    KH = [KH_UNIFORM] * tiles
    nseg, seg_tile_rng, lo_off, hi_off, seg_chunk_off, ncht = _chunk_layout(
        KL, KH, tiles, seg_tiles
    )
    return dict(
        lo_lim=LO_LIM,
        tiles=tiles,
        tpc=tpc,
        npad=C * tpc,
        nseg=nseg,
        seg_tile_rng=seg_tile_rng,
        seg_chunk_off=seg_chunk_off,
        KL=KL,
        KH=KH,
        lo_chunk_off=lo_off.tolist(),
        hi_chunk_off=hi_off.tolist(),
        ncht=ncht,
    )


def _blob_fields(st, wb1, wb2):
    """Packed int16 blob layout: name -> (offset, rows, i16cols)."""
    ncht = st["ncht"]
    tiles = st["tiles"]
    f = [
        ("w1", IN_CH, HID),
        ("w2", HID, OUT_CH),
        ("colv", P, ncht),
        ("idx", 16, ncht * 8),
        ("dinv", P, tiles * 2),
        ("iota", P, P),
    ]
    if wb1:
        f.append(("b1", P, HID * 2))
    if wb2:
        f.append(("b2", P, OUT_CH * 2))
    offs = {}
    o = 0
    for name, r, c in f:
        offs[name] = (o, r, c)
        o += r * c
    return offs, o


# ---------------------------------------------------------------- device prog


def build_program(st, with_bias1, with_bias2):
    import contextlib

    import concourse.bass as bass  # noqa: F401
    import concourse.tile as tile
    from concourse import bacc, mybir
    from concourse.masks import make_identity

    f32 = mybir.dt.float32
    bf16 = mybir.dt.bfloat16
    i16 = mybir.dt.int16
    AOT = mybir.AluOpType
    AF = mybir.ActivationFunctionType

    tiles = st["tiles"]
    tpc = st["tpc"]
    npad = st["npad"]
    nseg = st["nseg"]
    ncht = st["ncht"]
    KL, KH = st["KL"], st["KH"]
    lo_off, hi_off = st["lo_chunk_off"], st["hi_chunk_off"]

    nc = bacc.Bacc(None, target_bir_lowering=False)

    offs, blob_w = _blob_fields(st, with_bias1, with_bias2)
    xs_t_d = nc.dram_tensor("xs_t", [tpc, IN_CH], bf16, kind="ExternalInput")
    blob_d = nc.dram_tensor("blob", [blob_w], i16, kind="ExternalInput")
    out_d = nc.dram_tensor("out", [tpc, OUT_CH], bf16, kind="ExternalOutput")

    def blob_view(name):
        o, r, c = offs[name]
        return blob_d[o : o + r * c].rearrange("(r c) -> r c", c=c)

    with tile.TileContext(nc) as tc:
        with contextlib.ExitStack() as ctx:
            dram = ctx.enter_context(tc.tile_pool(name="dram", bufs=1, space="DRAM"))
            persist = ctx.enter_context(tc.tile_pool(name="persist", bufs=1))
            msg_pool = ctx.enter_context(tc.tile_pool(name="msg", bufs=2))
            s_pool = ctx.enter_context(tc.tile_pool(name="sel", bufs=8))
            tmp_pool = ctx.enter_context(tc.tile_pool(name="tmp", bufs=3))
            agg_ps = ctx.enter_context(tc.tile_pool(name="aggps", bufs=2, space="PSUM"))
            tp_ps = ctx.enter_context(tc.tile_pool(name="tpps", bufs=2, space="PSUM"))
            w_ps = ctx.enter_context(tc.tile_pool(name="wps", bufs=2, space="PSUM"))

            # persistent SBUF tensors
            xs_sb = persist.tile([P, tiles, IN_CH], bf16, name="xs_sb", tag="xs_sb")
            w1_sb = persist.tile([IN_CH, HID], bf16, name="w1_sb", tag="w1_sb")
            w2_sb = persist.tile([HID, OUT_CH], bf16, name="w2_sb", tag="w2_sb")
            if with_bias1:
                b1_sb = persist.tile([P, HID], f32, name="b1_sb", tag="b1_sb")
            if with_bias2:
                b2_sb = persist.tile([P, OUT_CH], f32, name="b2_sb", tag="b2_sb")
            colv_i16 = persist.tile([P, ncht], i16, name="colv_i16", tag="colv_i16")
            colv_sb = persist.tile([P, ncht], f32, name="colv_sb", tag="colv_sb")
            idx_sb = persist.tile([P, ncht * 8], i16, name="idx_sb", tag="idx_sb")
            dinv_sb = persist.tile([P, tiles], f32, name="dinv_sb", tag="dinv_sb")
            iota_bf = persist.tile([P, P], bf16, name="iota_bf", tag="iota_bf")
            ident_bf = persist.tile([P, P], bf16, name="ident_bf", tag="ident_bf")
            t1_own = persist.tile([P, tiles, HID], bf16, name="t1_own", tag="t1_own")
            t2_own = persist.tile([P, tiles, HID], bf16, name="t2_own", tag="t2_own")
            out_sb = persist.tile([P, tiles, OUT_CH], bf16, name="out_sb", tag="out_sb")

            xs_view = xs_t_d[:].rearrange("(t p) f -> p t f", p=P)
            nc.sync.dma_start(xs_sb[:], xs_view)
            nc.sync.dma_start(w1_sb[:].bitcast(i16), blob_view("w1"))
            nc.sync.dma_start(w2_sb[:].bitcast(i16), blob_view("w2"))
            if with_bias1:
                nc.sync.dma_start(b1_sb[:].bitcast(i16), blob_view("b1"))
            if with_bias2:
                nc.sync.dma_start(b2_sb[:].bitcast(i16), blob_view("b2"))
            nc.sync.dma_start(colv_i16[:], blob_view("colv"))
            nc.vector.tensor_copy(colv_sb[:], colv_i16[:])
            idx_view = blob_view("idx")
            for g in range(8):
                nc.sync.dma_start(idx_sb[g * 16 : (g + 1) * 16, :], idx_view)
            nc.sync.dma_start(dinv_sb[:].bitcast(i16), blob_view("dinv"))
            nc.sync.dma_start(iota_bf[:].bitcast(i16), blob_view("iota"))
            make_identity(nc, ident_bf[:])
            nc.gpsimd.memset(t2_own[:], 0.0)

            # DRAM tiles for collectives
            ag1_in = dram.tile([tpc, HID], bf16)
            table1 = dram.tile([npad, HID], bf16, addr_space="Shared")
            ag2_in = dram.tile([tpc, HID], bf16)
            table2 = dram.tile([npad, HID], bf16, addr_space="Shared")

            # ---- Phase A: table1 shard = (dinv*x) @ W1, allgather
            for t in range(tiles):
                xsc = tmp_pool.tile([P, IN_CH], bf16, tag="xsc")
                nc.scalar.activation(
                    xsc[:], xs_sb[:, t, :], AF.Copy, scale=dinv_sb[:, t : t + 1]
                )
                tx = tp_ps.tile([P, P], bf16, tag="tpT")
                nc.tensor.transpose(tx[:], xsc[:], ident_bf[:])
                xsT = tmp_pool.tile([P, P], bf16, tag="xsT")
                nc.scalar.activation(xsT[:], tx[:], AF.Copy)
                ps = tp_ps.tile([P, HID], f32, tag="tpA")
                nc.tensor.matmul(ps[:], xsT[:], w1_sb[:], start=True, stop=True)
                nc.scalar.activation(t1_own[:, t, :], ps[:], AF.Copy)

            ag1_view = ag1_in[:].rearrange("(t p) f -> p t f", p=P)
            nc.sync.dma_start(ag1_view, t1_own[:])
            nc.gpsimd.collective_compute(
                "AllGather",
                mybir.AluOpType.bypass,
                replica_groups=[list(range(C))],
                ins=[ag1_in[:].opt()],
                outs=[table1[:].opt()],
            )

            # ---- shared aggregation sweep
            def aggregate(table_d, n_out_ch, finish):
                for s in range(nseg):
                    t0, t1 = st["seg_tile_rng"][s]
                    off, nlo_c, nhi_c = st["seg_chunk_off"][s]
                    nch = nlo_c + nhi_c
                    msg = msg_pool.tile([P, nch, HID], bf16, tag="msg")
                    if nlo_c:
                        nc.gpsimd.dma_gather(
                            msg[:, :nlo_c, :],
                            table_d[:],
                            idx_sb[:, off * 8 : (off + nlo_c) * 8],
                            nlo_c * P,
                            nlo_c * P,
                            HID,
                            single_packet=False,
                        )
                    if nhi_c:
                        nc.gpsimd.dma_gather(
                            msg[:, nlo_c:, :],
                            table_d[st["lo_lim"] :, :],
                            idx_sb[:, (off + nlo_c) * 8 : (off + nch) * 8],
                            nhi_c * P,
                            nhi_c * P,
                            HID,
                            single_packet=False,
                        )
                    for t in range(t0, t1):
                        ks = [lo_off[t] + j for j in range(KL[t])] + [
                            hi_off[t] + j for j in range(KH[t])
                        ]
                        ps = agg_ps.tile([P, n_out_ch], f32, tag="agg")
                        for j, gk in enumerate(ks):
                            S = s_pool.tile([P, P], bf16, tag="sel")
                            nc.vector.tensor_scalar(
                                S[:],
                                iota_bf[:],
                                colv_sb[:, gk : gk + 1],
                                0.0,
                                op0=AOT.subtract,
                                op1=AOT.is_equal,
                            )
                            nc.tensor.matmul(
                                ps[:],
                                S[:],
                                msg[:, gk - off, :n_out_ch],
                                start=(j == 0),
                                stop=(j == len(ks) - 1),
                            )
                        finish(t, ps)

            # ---- Phase B: layer-1 epilogue builds table2 shard
            def finish1(t, ps):
                tmp = tmp_pool.tile([P, HID], f32, tag="tmp")
                nc.vector.tensor_tensor(tmp[:], ps[:], t1_own[:, t, :], op=AOT.add)
                if with_bias1:
                    hs = tmp_pool.tile([P, HID], f32, tag="hs")
                    nc.scalar.activation(
                        hs[:], tmp[:], AF.Copy, scale=dinv_sb[:, t : t + 1]
                    )
                    nc.vector.tensor_tensor(hs[:], hs[:], b1_sb[:], op=AOT.add)
                    # relu(dinv*z) == dinv*relu(z) since dinv >= 0
                    hr = tmp_pool.tile([P, HID], f32, tag="hr")
                    nc.scalar.activation(
                        hr[:], hs[:], AF.Relu, scale=dinv_sb[:, t : t + 1]
                    )
                else:
                    hr0 = tmp_pool.tile([P, HID], f32, tag="hs")
                    nc.scalar.activation(
                        hr0[:], tmp[:], AF.Relu, scale=dinv_sb[:, t : t + 1]
                    )
                    hr = tmp_pool.tile([P, HID], f32, tag="hr")
                    nc.scalar.activation(
                        hr[:], hr0[:], AF.Copy, scale=dinv_sb[:, t : t + 1]
                    )
                t2pre = tmp_pool.tile([P, HID], bf16, tag="t2pre")
                nc.vector.tensor_copy(t2pre[:], hr[:])
                tp = tp_ps.tile([P, P], bf16, tag="tpT")
                nc.tensor.transpose(tp[:], t2pre[:], ident_bf[:])
                t2T = tmp_pool.tile([P, P], bf16, tag="t2T")
                nc.scalar.activation(t2T[:], tp[:], AF.Copy)
                ps2 = w_ps.tile([P, OUT_CH], f32, tag="w")
                nc.tensor.matmul(ps2[:], t2T[:], w2_sb[:], start=True, stop=True)
                nc.scalar.activation(t2_own[:, t, :OUT_CH], ps2[:], AF.Copy)

            aggregate(table1, HID, finish1)

            ag2_view = ag2_in[:].rearrange("(t p) f -> p t f", p=P)
            nc.sync.dma_start(ag2_view, t2_own[:])
            nc.gpsimd.collective_compute(
                "AllGather",
                mybir.AluOpType.bypass,
                replica_groups=[list(range(C))],
                ins=[ag2_in[:].opt()],
                outs=[table2[:].opt()],
            )

            # ---- Phase C: layer-2 epilogue writes output
            def finish2(t, ps):
                tmp = tmp_pool.tile([P, OUT_CH], f32, tag="tmp2")
                nc.vector.tensor_tensor(
                    tmp[:], ps[:], t2_own[:, t, :OUT_CH], op=AOT.add
                )
                if with_bias2:
                    o1 = tmp_pool.tile([P, OUT_CH], f32, tag="o1")
                    nc.scalar.activation(
                        o1[:], tmp[:], AF.Copy, scale=dinv_sb[:, t : t + 1]
                    )
                    nc.vector.tensor_tensor(
                        out_sb[:, t, :], o1[:], b2_sb[:], op=AOT.add
                    )
                else:
                    nc.scalar.activation(
                        out_sb[:, t, :], tmp[:], AF.Copy, scale=dinv_sb[:, t : t + 1]
                    )

            aggregate(table2, OUT_CH, finish2)

            out_view = out_d[:].rearrange("(t p) f -> p t f", p=P)
            nc.sync.dma_start(out_view, out_sb[:])

    nc.compile()
    return nc


# ---------------------------------------------------------------- runner


class _Runner:
    """Holds the jitted shard_map callable + device-resident output-init bufs."""

    def __init__(self, nc):
        import jax
        from jax.experimental.shard_map import shard_map
        from jax.sharding import Mesh, NamedSharding, PartitionSpec

        from concourse import bass2jax, mybir

        bass2jax.install_neuronx_cc_hook()
        partition_name = (
            nc.partition_id_tensor.name if nc.partition_id_tensor else None
        )

        in_names, out_names, out_avals, zero_specs = [], [], [], []
        for alloc in nc.m.functions[0].allocations:
            if not isinstance(alloc, mybir.MemoryLocationSet):
                continue
            name = alloc.memorylocations[0].name
            if alloc.kind == "ExternalInput":
                if name != partition_name:
                    in_names.append(name)
            elif alloc.kind == "ExternalOutput":
                shape = tuple(alloc.tensor_shape)
                dtype = mybir.dt.np(alloc.dtype)
                out_names.append(name)
                out_avals.append(jax.core.ShapedArray(shape, dtype))
                zero_specs.append((shape, dtype))
        n_params = len(in_names)
        n_outs = len(out_names)
        all_in_names = list(in_names) + list(out_names)
        if partition_name is not None:
            all_in_names.append(partition_name)

        def _body(*args):
            operands = list(args)
            if partition_name is not None:
                operands.append(bass2jax.partition_id_tensor())
            outs = bass2jax._bass_exec_p.bind(
                *operands,
                out_avals=tuple(out_avals),
                in_names=tuple(all_in_names),
                out_names=tuple(out_names),
                lowering_input_output_aliases=(),
                sim_require_finite=True,
                sim_require_nnan=True,
                nc=nc,
            )
            return tuple(outs)

        devices = jax.devices()[:C]
        mesh = Mesh(np.asarray(devices), ("core",))
        in_specs = (PartitionSpec("core"),) * (n_params + n_outs)
        out_specs = (PartitionSpec("core"),) * n_outs
        self.sharded = jax.jit(
            shard_map(
                _body,
                mesh=mesh,
                in_specs=in_specs,
                out_specs=out_specs,
                check_rep=False,
            ),
            keep_unused=True,
        )
        self.sh = NamedSharding(mesh, PartitionSpec("core"))
        self.in_names = in_names
        self.out_names = out_names
        self.zero_specs = zero_specs
        # the kernel writes every output element, so these never matter
        self.zeros_dev = [
            jax.device_put(np.zeros((C * s[0], *s[1:]), d), self.sh)
            for (s, d) in zero_specs
        ]
        self.jax = jax

    def device_put_async(self, arr):
        return self.jax.device_put(arr, self.sh)

    def __call__(self, *concat_inputs):
        out_arrs = self.sharded(*concat_inputs, *self.zeros_dev)
        return [
            np.asarray(out_arrs[i]).reshape(C, *self.zero_specs[i][0])
            for i in range(len(self.out_names))
        ]


def _get_runner(st, with_bias1, with_bias2):
    key = (
        st["tpc"],
        st["nseg"],
        st["lo_lim"],
        st["ncht"],
        tuple(st["KL"]),
        tuple(st["KH"]),
        with_bias1,
        with_bias2,
    )
    r = _runner_cache.get(key)
    if r is None:
        nc = build_program(st, with_bias1, with_bias2)
        r = _Runner(nc)
        _runner_cache[key] = r
    return r


def _fill_blob_weights(blob, st, W1, b1, W2, b2, wb1, wb2):
    offs, _ = _blob_fields(st, wb1, wb2)
    o, r, c = offs["w1"]
    blob[:, o : o + r * c] = (
        np.asarray(W1, np.float32).astype(ml_dtypes.bfloat16).view(np.int16).reshape(-1)
    )[None, :]
    o, r, c = offs["w2"]
    blob[:, o : o + r * c] = (
        np.asarray(W2, np.float32).astype(ml_dtypes.bfloat16).view(np.int16).reshape(-1)
    )[None, :]
    if wb1:
        o, r, c = offs["b1"]
        b1_t = np.tile(np.asarray(b1, np.float32), (P, 1))
        blob[:, o : o + r * c] = b1_t.view(np.int16).reshape(-1)[None, :]
    if wb2:
        o, r, c = offs["b2"]
        b2_t = np.tile(np.asarray(b2, np.float32), (P, 1))
        blob[:, o : o + r * c] = b2_t.view(np.int16).reshape(-1)[None, :]


def _pack_xs(x, n_nodes, npad):
    xs = np.zeros((npad, IN_CH), dtype=ml_dtypes.bfloat16)
    xs[:n_nodes] = x.astype(ml_dtypes.bfloat16)
    return xs


# ---------------------------------------------------------------- warmup

_warm_ready = False


def _warmup():
    global _warm_ready
    if os.environ.get("GCN_NO_PREBUILD"):
        return
    try:
        st = _uniform_st()
        runner = _get_runner(st, False, False)
        _, blob_w = _blob_fields(st, False, False)
        xs = np.zeros((C * TPC, IN_CH), ml_dtypes.bfloat16)
        blob = np.zeros((C * blob_w,), np.int16)
        runner(xs, blob)
        # warm the exact real-call path too (device-resident xs arg)
        xs_dev = runner.device_put_async(xs)
        runner(xs_dev, blob)
        _warm_ready = True
    except Exception:
        import traceback

        traceback.print_exc()
        _warm_ready = False


# ---------------------------------------------------------------- entry


def _run(x, edge_index, W1, b1, W2, b2, n_nodes, tpc, seg_tiles, lo_lim=LO_LIM,
         use_sim=False, uniform=False):
    wb1 = bool(np.any(np.asarray(b1) != 0))
    wb2 = bool(np.any(np.asarray(b2) != 0))
    extra = dict(wb1=wb1, wb2=wb2)

    runner = None
    xs_dev = None
    if uniform and not use_sim and not wb1 and not wb2:
        # start the (big) feature upload before edge preprocessing
        try:
            runner = _get_runner(_uniform_st(tpc, seg_tiles), False, False)
            xs_dev = runner.device_put_async(_pack_xs(x, n_nodes, C * tpc))
        except Exception:
            runner = None

    KL = KH = None
    if uniform and not wb1 and not wb2:
        KL, KH = KL_UNIFORM, KH_UNIFORM
    st, blob, fits = _preprocess(
        x, edge_index, n_nodes, tpc, seg_tiles, lo_lim, KL, KH, extra
    )
    if not fits:  # caps exceeded -> data-driven structure
        st, blob, _ = _preprocess(
            x, edge_index, n_nodes, tpc, seg_tiles, lo_lim, None, None, extra
        )
        runner = None
        xs_dev = None

    _fill_blob_weights(blob, st, W1, b1, W2, b2, wb1, wb2)

    if use_sim:
        from concourse import bass_interp

        key = ("sim", st["ncht"], tuple(st["KL"]), tuple(st["KH"]), wb1, wb2)
        nc = _prog_cache.get(key)
        if nc is None:
            nc = build_program(st, wb1, wb2)
            _prog_cache[key] = nc
        xs = _pack_xs(x, n_nodes, C * tpc).reshape(C, tpc, IN_CH)
        sim = bass_interp.MultiCoreSim(nc, C)
        for c in range(C):
            sim.cores[c].tensor("xs_t")[:] = xs[c]
            sim.cores[c].tensor("blob")[:] = blob[c]
        sim.simulate()
        outs = np.stack([sim.cores[c].mem_tensor("out") for c in range(C)])
    else:
        if runner is None:
            runner = _get_runner(st, wb1, wb2)
        if xs_dev is None:
            xs_dev = _pack_xs(x, n_nodes, C * tpc)
        outs = runner(xs_dev, blob.reshape(-1))[0]

    full = outs.reshape(C * tpc, OUT_CH)[:n_nodes]
    return np.asarray(full, dtype=np.float32)


def _gcn_host(x, edge_index, W1, b1, W2, b2):
    """Pure-numpy fallback (used only if the device path fails)."""
    n = x.shape[0]
    row = edge_index[0].astype(np.int64)
    col = edge_index[1].astype(np.int64)
    loops = np.arange(n, dtype=np.int64)
    row_f = np.concatenate([row, loops])
    col_f = np.concatenate([col, loops])
    deg = np.bincount(col_f, minlength=n).astype(np.float32)
    dinv = np.where(deg > 0, 1.0 / np.sqrt(deg), 0.0).astype(np.float32)
    norm = (dinv[row_f] * dinv[col_f]).astype(np.float32)
    order = np.argsort(col_f, kind="stable")
    row_sv = row_f[order]
    col_sv = col_f[order]
    norm_sv = norm[order][:, None]
    starts = np.searchsorted(col_sv, np.arange(n, dtype=np.int64))

    def conv(h, W, b):
        msg = norm_sv * (h @ W)[row_sv]
        return np.add.reduceat(msg, starts, axis=0) + b

    h = np.maximum(conv(x, W1, b1), 0.0)
    return conv(h, W2, b2).astype(np.float32)


def kernel(x, edge_index, W1, b1, W2, b2):
    x = np.asarray(x, dtype=np.float32)
    edge_index = np.asarray(edge_index)
    assert x.shape[0] == N_NODES
    args = (
        x,
        edge_index,
        np.asarray(W1, np.float32),
        np.asarray(b1, np.float32),
        np.asarray(W2, np.float32),
        np.asarray(b2, np.float32),
    )
    try:
        return _run(*args, N_NODES, tpc=TPC, seg_tiles=SEG_TILES, uniform=True)
    except Exception:
        import traceback

        traceback.print_exc()
        return _gcn_host(*args)


_warmup()


# revision 23
# speedup vs baseline: 13.9621x; 1.7170x over previous
"""GCN encoder (2-layer GCNConv, PyG-default normalization) on 8 trn2 NeuronCores.

Full inputs in, full output out. Internally:
  - nodes are padded to NPAD = 8*TPC and sharded contiguously across 8 cores
  - normalization is folded into the gather tables:
      table1 = (dinv*x) @ W1          (bf16, AllGathered to every core)
      conv1  = dinv * (segsum_edges(table1[row]) + table1[own]) + b1
      table2 = (dinv * relu(conv1)) @ W2   (bf16, padded to 128 cols, AllGathered)
      out    = dinv * (segsum_edges(table2[row]) + table2[own]) + b2
  - per-edge gathers run on-device via gpsimd dma_gather (int16 indices,
    lo/hi split around 32768); segment sums are TensorE matmuls against
    0/1 selection matrices built on VectorE from edge target values.

One SPMD program is prebuilt at import time with a data-independent uniform
chunk structure (per-tile lo/hi chunk counts sized for the random-graph tail)
and warmed up with a dummy run, so the real call only pays preprocessing +
upload + execute. If the actual data exceeds the uniform caps (or biases are
nonzero), a data-driven program is built on the fly instead.
"""

import math
import os

import numpy as np
import ml_dtypes

P = 128
C = 8  # cores
N_NODES = 50000
IN_CH = 128
HID = 128
OUT_CH = 64
LO_LIM = 32768

TPC = 6272  # targets per core; 49 tiles of 128
TILES = TPC // P
SEG_TILES = 7
# uniform per-tile chunk caps: lo edges/tile ~ Poisson(1337) sd 37 -> 13*128=1664
# hi ~ Poisson(704) sd 27 -> 7*128=896
KL_UNIFORM = 13
KH_UNIFORM = 7

_prog_cache = {}
_runner_cache = {}


# ---------------------------------------------------------------- host prep


def _chunk_layout(KL, KH, tiles, seg_tiles):
    nseg = math.ceil(tiles / seg_tiles)
    seg_tile_rng = [
        (s * seg_tiles, min((s + 1) * seg_tiles, tiles)) for s in range(nseg)
    ]
    lo_chunk_off = np.zeros(tiles, dtype=np.int64)
    hi_chunk_off = np.zeros(tiles, dtype=np.int64)
    seg_chunk_off = []
    k = 0
    for t0, t1 in seg_tile_rng:
        off = k
        for t in range(t0, t1):
            lo_chunk_off[t] = k
            k += KL[t]
        nlo_c = k - off
        for t in range(t0, t1):
            hi_chunk_off[t] = k
            k += KH[t]
        seg_chunk_off.append((off, int(nlo_c), int(k - off - nlo_c)))
    return nseg, seg_tile_rng, lo_chunk_off, hi_chunk_off, seg_chunk_off, int(k)


def _preprocess(x, edge_index, n_nodes, tpc, seg_tiles, lo_lim=LO_LIM, KL=None, KH=None, st_extra=None):
    """Returns (st, blob, fits). When KL/KH given (uniform caps), `fits`
    reports whether the data obeys them; otherwise caps are data-driven."""
    tiles = tpc // P
    npad = C * tpc
    n_gt = C * tiles

    row = np.ascontiguousarray(edge_index[0], dtype=np.int32)
    col = np.ascontiguousarray(edge_index[1], dtype=np.int32)

    deg = np.bincount(col, minlength=npad).astype(np.float32) + 1.0
    deg[n_nodes:] = 0.0
    dinv = np.zeros(npad, dtype=np.float32)
    dinv[:n_nodes] = 1.0 / np.sqrt(deg[:n_nodes])

    is_lo = row < lo_lim
    bucket = ((col >> 7) << 1) | is_lo  # gtile*2 + is_lo
    counts = np.bincount(bucket, minlength=n_gt * 2).reshape(C, tiles, 2)
    n_hi = counts[:, :, 0]
    n_lo = counts[:, :, 1]

    fits = True
    if KL is None:
        KL = np.maximum(1, np.ceil(n_lo.max(axis=0) / P).astype(np.int64)).tolist()
        KH = np.ceil(n_hi.max(axis=0) / P).astype(np.int64).tolist()
    else:
        KL = [KL] * tiles if np.isscalar(KL) else list(KL)
        KH = [KH] * tiles if np.isscalar(KH) else list(KH)
        if (n_lo.max(axis=0) > np.array(KL) * P).any() or (
            n_hi.max(axis=0) > np.array(KH) * P
        ).any():
            fits = False

    nseg, seg_tile_rng, lo_off, hi_off, seg_chunk_off, ncht = _chunk_layout(
        KL, KH, tiles, seg_tiles
    )

    wb1 = bool(st_extra.get("wb1")) if st_extra else False
    wb2 = bool(st_extra.get("wb2")) if st_extra else False
    offs, blob_w = _blob_fields(dict(tiles=tiles, ncht=ncht), wb1, wb2)
    blob = np.empty((C, blob_w), dtype=np.int16)

    if fits:
        order = np.argsort(bucket.astype(np.int16), kind="stable")
        b_s = bucket[order]
        bstart = np.searchsorted(b_s, np.arange(n_gt * 2, dtype=np.int32))
        pos = np.arange(len(b_s), dtype=np.int32) - bstart[b_s].astype(np.int32)
        col_s = col[order]
        row_s = row[order]
        lo_s = is_lo[order]
        # per-bucket slot base in the global [C * ncht * P] edge-slot space
        b_ids = np.arange(n_gt * 2, dtype=np.int32)
        gt = b_ids >> 1
        core_of = gt // tiles
        t_of = gt % tiles
        base_b = core_of * (ncht * P) + np.where(
            b_ids & 1, lo_off[t_of] * P, hi_off[t_of] * P
        ).astype(np.int64)
        slot = base_b[b_s] + pos
        cv_all = np.full(C * ncht * P, -1, dtype=np.int16)
        ix_all = np.zeros(C * ncht * P, dtype=np.int16)
        cv_all[slot] = (col_s & 127).astype(np.int16)
        ix_all[slot] = np.where(lo_s, row_s, row_s - lo_lim).astype(np.int16)
        o, rr, cc = offs["colv"]
        blob[:, o : o + rr * cc] = (
            cv_all.reshape(C, ncht, P).transpose(0, 2, 1).reshape(C, -1)
        )
        o, rr, cc = offs["idx"]
        blob[:, o : o + rr * cc] = (
            ix_all.reshape(C, ncht * 8, 16).transpose(0, 2, 1).reshape(C, -1)
        )

    dinv_sb = dinv.reshape(C, tiles, P).transpose(0, 2, 1)
    o, rr, cc = offs["dinv"]
    blob[:, o : o + rr * cc] = (
        np.ascontiguousarray(dinv_sb).view(np.int16).reshape(C, -1)
    )
    iota = np.tile(np.arange(P, dtype=np.float32), (P, 1)).astype(ml_dtypes.bfloat16)
    o, rr, cc = offs["iota"]
    blob[:, o : o + rr * cc] = iota.view(np.int16).reshape(-1)[None, :]

    st = dict(
        lo_lim=lo_lim,
        tiles=tiles,
        tpc=tpc,
        npad=npad,
        nseg=nseg,
        seg_tile_rng=seg_tile_rng,
        seg_chunk_off=seg_chunk_off,
        KL=list(KL),
        KH=list(KH),
        lo_chunk_off=lo_off.tolist(),
        hi_chunk_off=hi_off.tolist(),
        ncht=ncht,
    )
    return st, blob, fits


def _uniform_st(tpc=TPC, seg_tiles=SEG_TILES):
    tiles = tpc // P
    KL = [KL_UNIFORM] * tiles
    KH = [KH_UNIFORM] * tiles
    nseg, seg_tile_rng, lo_off, hi_off, seg_chunk_off, ncht = _chunk_layout(
        KL, KH, tiles, seg_tiles
    )
    return dict(
        lo_lim=LO_LIM,
        tiles=tiles,
        tpc=tpc,
        npad=C * tpc,
        nseg=nseg,
        seg_tile_rng=seg_tile_rng,
        seg_chunk_off=seg_chunk_off,
        KL=KL,
        KH=KH,
        lo_chunk_off=lo_off.tolist(),
        hi_chunk_off=hi_off.tolist(),
        ncht=ncht,
    )


def _blob_fields(st, wb1, wb2):
    """Packed int16 blob layout: name -> (offset, rows, i16cols)."""
    ncht = st["ncht"]
    tiles = st["tiles"]
    f = [
        ("w1", IN_CH, HID),
        ("w2", HID, OUT_CH),
        ("colv", P, ncht),
        ("idx", 16, ncht * 8),
        ("dinv", P, tiles * 2),
        ("iota", P, P),
    ]
    if wb1:
        f.append(("b1", P, HID * 2))
    if wb2:
        f.append(("b2", P, OUT_CH * 2))
    offs = {}
    o = 0
    for name, r, c in f:
        offs[name] = (o, r, c)
        o += r * c
    return offs, o


# ---------------------------------------------------------------- device prog


def build_program(st, with_bias1, with_bias2):
    import contextlib

    import concourse.bass as bass  # noqa: F401
    import concourse.tile as tile
    from concourse import bacc, mybir
    from concourse.masks import make_identity

    f32 = mybir.dt.float32
    bf16 = mybir.dt.bfloat16
    i16 = mybir.dt.int16
    AOT = mybir.AluOpType
    AF = mybir.ActivationFunctionType

    tiles = st["tiles"]
    tpc = st["tpc"]
    npad = st["npad"]
    nseg = st["nseg"]
    ncht = st["ncht"]
    KL, KH = st["KL"], st["KH"]
    lo_off, hi_off = st["lo_chunk_off"], st["hi_chunk_off"]

    nc = bacc.Bacc(None, target_bir_lowering=False)

    offs, blob_w = _blob_fields(st, with_bias1, with_bias2)
    xs_t_d = nc.dram_tensor("xs_t", [tpc, IN_CH], bf16, kind="ExternalInput")
    blob_d = nc.dram_tensor("blob", [blob_w], i16, kind="ExternalInput")
    out_d = nc.dram_tensor("out", [tpc, OUT_CH], bf16, kind="ExternalOutput")

    def blob_view(name):
        o, r, c = offs[name]
        return blob_d[o : o + r * c].rearrange("(r c) -> r c", c=c)

    with tile.TileContext(nc) as tc:
        with contextlib.ExitStack() as ctx:
            dram = ctx.enter_context(tc.tile_pool(name="dram", bufs=1, space="DRAM"))
            persist = ctx.enter_context(tc.tile_pool(name="persist", bufs=1))
            msg_pool = ctx.enter_context(tc.tile_pool(name="msg", bufs=2))
            s_pool = ctx.enter_context(tc.tile_pool(name="sel", bufs=8))
            tmp_pool = ctx.enter_context(tc.tile_pool(name="tmp", bufs=3))
            agg_ps = ctx.enter_context(tc.tile_pool(name="aggps", bufs=2, space="PSUM"))
            tp_ps = ctx.enter_context(tc.tile_pool(name="tpps", bufs=2, space="PSUM"))
            w_ps = ctx.enter_context(tc.tile_pool(name="wps", bufs=2, space="PSUM"))

            # persistent SBUF tensors
            xs_sb = persist.tile([P, tiles, IN_CH], bf16, name="xs_sb", tag="xs_sb")
            w1_sb = persist.tile([IN_CH, HID], bf16, name="w1_sb", tag="w1_sb")
            w2_sb = persist.tile([HID, OUT_CH], bf16, name="w2_sb", tag="w2_sb")
            if with_bias1:
                b1_sb = persist.tile([P, HID], f32, name="b1_sb", tag="b1_sb")
            if with_bias2:
                b2_sb = persist.tile([P, OUT_CH], f32, name="b2_sb", tag="b2_sb")
            colv_i16 = persist.tile([P, ncht], i16, name="colv_i16", tag="colv_i16")
            colv_sb = persist.tile([P, ncht], f32, name="colv_sb", tag="colv_sb")
            idx_sb = persist.tile([P, ncht * 8], i16, name="idx_sb", tag="idx_sb")
            dinv_sb = persist.tile([P, tiles], f32, name="dinv_sb", tag="dinv_sb")
            iota_bf = persist.tile([P, P], bf16, name="iota_bf", tag="iota_bf")
            ident_bf = persist.tile([P, P], bf16, name="ident_bf", tag="ident_bf")
            t1_own = persist.tile([P, tiles, HID], bf16, name="t1_own", tag="t1_own")
            t2_own = persist.tile([P, tiles, HID], bf16, name="t2_own", tag="t2_own")
            out_sb = persist.tile([P, tiles, OUT_CH], bf16, name="out_sb", tag="out_sb")

            xs_view = xs_t_d[:].rearrange("(t p) f -> p t f", p=P)
            nc.sync.dma_start(xs_sb[:], xs_view)
            nc.sync.dma_start(w1_sb[:].bitcast(i16), blob_view("w1"))
            nc.sync.dma_start(w2_sb[:].bitcast(i16), blob_view("w2"))
            if with_bias1:
                nc.sync.dma_start(b1_sb[:].bitcast(i16), blob_view("b1"))
            if with_bias2:
                nc.sync.dma_start(b2_sb[:].bitcast(i16), blob_view("b2"))
            nc.sync.dma_start(colv_i16[:], blob_view("colv"))
            nc.vector.tensor_copy(colv_sb[:], colv_i16[:])
            idx_view = blob_view("idx")
            for g in range(8):
                nc.sync.dma_start(idx_sb[g * 16 : (g + 1) * 16, :], idx_view)
            nc.sync.dma_start(dinv_sb[:].bitcast(i16), blob_view("dinv"))
            nc.sync.dma_start(iota_bf[:].bitcast(i16), blob_view("iota"))
            make_identity(nc, ident_bf[:])
            nc.gpsimd.memset(t2_own[:], 0.0)

            # DRAM tiles for collectives
            ag1_in = dram.tile([tpc, HID], bf16)
            table1 = dram.tile([npad, HID], bf16, addr_space="Shared")
            ag2_in = dram.tile([tpc, HID], bf16)
            table2 = dram.tile([npad, HID], bf16, addr_space="Shared")

            # ---- Phase A: table1 shard = (dinv*x) @ W1, allgather
            for t in range(tiles):
                xsc = tmp_pool.tile([P, IN_CH], bf16, tag="xsc")
                nc.scalar.activation(
                    xsc[:], xs_sb[:, t, :], AF.Copy, scale=dinv_sb[:, t : t + 1]
                )
                tx = tp_ps.tile([P, P], bf16, tag="tpT")
                nc.tensor.transpose(tx[:], xsc[:], ident_bf[:])
                xsT = tmp_pool.tile([P, P], bf16, tag="xsT")
                nc.scalar.activation(xsT[:], tx[:], AF.Copy)
                ps = tp_ps.tile([P, HID], f32, tag="tpA")
                nc.tensor.matmul(ps[:], xsT[:], w1_sb[:], start=True, stop=True)
                nc.scalar.activation(t1_own[:, t, :], ps[:], AF.Copy)

            ag1_view = ag1_in[:].rearrange("(t p) f -> p t f", p=P)
            nc.sync.dma_start(ag1_view, t1_own[:])
            nc.gpsimd.collective_compute(
                "AllGather",
                mybir.AluOpType.bypass,
                replica_groups=[list(range(C))],
                ins=[ag1_in[:].opt()],
                outs=[table1[:].opt()],
            )

            # ---- shared aggregation sweep
            def aggregate(table_d, n_out_ch, finish):
                for s in range(nseg):
                    t0, t1 = st["seg_tile_rng"][s]
                    off, nlo_c, nhi_c = st["seg_chunk_off"][s]
                    nch = nlo_c + nhi_c
                    msg = msg_pool.tile([P, nch, HID], bf16, tag="msg")
                    if nlo_c:
                        nc.gpsimd.dma_gather(
                            msg[:, :nlo_c, :],
                            table_d[:],
                            idx_sb[:, off * 8 : (off + nlo_c) * 8],
                            nlo_c * P,
                            nlo_c * P,
                            HID,
                            single_packet=False,
                        )
                    if nhi_c:
                        nc.gpsimd.dma_gather(
                            msg[:, nlo_c:, :],
                            table_d[st["lo_lim"] :, :],
                            idx_sb[:, (off + nlo_c) * 8 : (off + nch) * 8],
                            nhi_c * P,
                            nhi_c * P,
                            HID,
                            single_packet=False,
                        )
                    for t in range(t0, t1):
                        ks = [lo_off[t] + j for j in range(KL[t])] + [
                            hi_off[t] + j for j in range(KH[t])
                        ]
                        ps = agg_ps.tile([P, n_out_ch], f32, tag="agg")
                        for j, gk in enumerate(ks):
                            S = s_pool.tile([P, P], bf16, tag="sel")
                            nc.vector.tensor_scalar(
                                S[:],
                                iota_bf[:],
                                colv_sb[:, gk : gk + 1],
                                0.0,
                                op0=AOT.subtract,
                                op1=AOT.is_equal,
                            )
                            nc.tensor.matmul(
                                ps[:],
                                S[:],
                                msg[:, gk - off, :n_out_ch],
                                start=(j == 0),
                                stop=(j == len(ks) - 1),
                            )
                        finish(t, ps)

            # ---- Phase B: layer-1 epilogue builds table2 shard
            def finish1(t, ps):
                tmp = tmp_pool.tile([P, HID], f32, tag="tmp")
                nc.vector.tensor_tensor(tmp[:], ps[:], t1_own[:, t, :], op=AOT.add)
                if with_bias1:
                    hs = tmp_pool.tile([P, HID], f32, tag="hs")
                    nc.scalar.activation(
                        hs[:], tmp[:], AF.Copy, scale=dinv_sb[:, t : t + 1]
                    )
                    nc.vector.tensor_tensor(hs[:], hs[:], b1_sb[:], op=AOT.add)
                    # relu(dinv*z) == dinv*relu(z) since dinv >= 0
                    hr = tmp_pool.tile([P, HID], f32, tag="hr")
                    nc.scalar.activation(
                        hr[:], hs[:], AF.Relu, scale=dinv_sb[:, t : t + 1]
                    )
                else:
                    hr0 = tmp_pool.tile([P, HID], f32, tag="hs")
                    nc.scalar.activation(
                        hr0[:], tmp[:], AF.Relu, scale=dinv_sb[:, t : t + 1]
                    )
                    hr = tmp_pool.tile([P, HID], f32, tag="hr")
                    nc.scalar.activation(
                        hr[:], hr0[:], AF.Copy, scale=dinv_sb[:, t : t + 1]
                    )
                t2pre = tmp_pool.tile([P, HID], bf16, tag="t2pre")
                nc.vector.tensor_copy(t2pre[:], hr[:])
                tp = tp_ps.tile([P, P], bf16, tag="tpT")
                nc.tensor.transpose(tp[:], t2pre[:], ident_bf[:])
                t2T = tmp_pool.tile([P, P], bf16, tag="t2T")
                nc.scalar.activation(t2T[:], tp[:], AF.Copy)
                ps2 = w_ps.tile([P, OUT_CH], f32, tag="w")
                nc.tensor.matmul(ps2[:], t2T[:], w2_sb[:], start=True, stop=True)
                nc.scalar.activation(t2_own[:, t, :OUT_CH], ps2[:], AF.Copy)

            aggregate(table1, HID, finish1)

            ag2_view = ag2_in[:].rearrange("(t p) f -> p t f", p=P)
            nc.sync.dma_start(ag2_view, t2_own[:])
            nc.gpsimd.collective_compute(
                "AllGather",
                mybir.AluOpType.bypass,
                replica_groups=[list(range(C))],
                ins=[ag2_in[:].opt()],
                outs=[table2[:].opt()],
            )

            # ---- Phase C: layer-2 epilogue writes output
            def finish2(t, ps):
                tmp = tmp_pool.tile([P, OUT_CH], f32, tag="tmp2")
                nc.vector.tensor_tensor(
                    tmp[:], ps[:], t2_own[:, t, :OUT_CH], op=AOT.add
                )
                if with_bias2:
                    o1 = tmp_pool.tile([P, OUT_CH], f32, tag="o1")
                    nc.scalar.activation(
                        o1[:], tmp[:], AF.Copy, scale=dinv_sb[:, t : t + 1]
                    )
                    nc.vector.tensor_tensor(
                        out_sb[:, t, :], o1[:], b2_sb[:], op=AOT.add
                    )
                else:
                    nc.scalar.activation(
                        out_sb[:, t, :], tmp[:], AF.Copy, scale=dinv_sb[:, t : t + 1]
                    )

            aggregate(table2, OUT_CH, finish2)

            out_view = out_d[:].rearrange("(t p) f -> p t f", p=P)
            nc.sync.dma_start(out_view, out_sb[:])

    nc.compile()
    return nc


# ---------------------------------------------------------------- runner


class _Runner:
    """Holds the jitted shard_map callable + device-resident output-init bufs."""

    def __init__(self, nc):
        import jax
        from jax.experimental.shard_map import shard_map
        from jax.sharding import Mesh, NamedSharding, PartitionSpec

        from concourse import bass2jax, mybir

        bass2jax.install_neuronx_cc_hook()
        partition_name = (
            nc.partition_id_tensor.name if nc.partition_id_tensor else None
        )

        in_names, out_names, out_avals, zero_specs = [], [], [], []
        for alloc in nc.m.functions[0].allocations:
            if not isinstance(alloc, mybir.MemoryLocationSet):
                continue
            name = alloc.memorylocations[0].name
            if alloc.kind == "ExternalInput":
                if name != partition_name:
                    in_names.append(name)
            elif alloc.kind == "ExternalOutput":
                shape = tuple(alloc.tensor_shape)
                dtype = mybir.dt.np(alloc.dtype)
                out_names.append(name)
                out_avals.append(jax.core.ShapedArray(shape, dtype))
                zero_specs.append((shape, dtype))
        n_params = len(in_names)
        n_outs = len(out_names)
        all_in_names = list(in_names) + list(out_names)
        if partition_name is not None:
            all_in_names.append(partition_name)

        def _body(*args):
            operands = list(args)
            if partition_name is not None:
                operands.append(bass2jax.partition_id_tensor())
            outs = bass2jax._bass_exec_p.bind(
                *operands,
                out_avals=tuple(out_avals),
                in_names=tuple(all_in_names),
                out_names=tuple(out_names),
                lowering_input_output_aliases=(),
                sim_require_finite=True,
                sim_require_nnan=True,
                nc=nc,
            )
            return tuple(outs)

        devices = jax.devices()[:C]
        mesh = Mesh(np.asarray(devices), ("core",))
        in_specs = (PartitionSpec("core"),) * (n_params + n_outs)
        out_specs = (PartitionSpec("core"),) * n_outs
        self.sharded = jax.jit(
            shard_map(
                _body,
                mesh=mesh,
                in_specs=in_specs,
                out_specs=out_specs,
                check_rep=False,
            ),
            keep_unused=True,
        )
        self.sh = NamedSharding(mesh, PartitionSpec("core"))
        self.in_names = in_names
        self.out_names = out_names
        self.zero_specs = zero_specs
        # the kernel writes every output element, so these never matter
        self.zeros_dev = [
            jax.device_put(np.zeros((C * s[0], *s[1:]), d), self.sh)
            for (s, d) in zero_specs
        ]
        self.jax = jax

    def device_put_async(self, arr):
        return self.jax.device_put(arr, self.sh)

    def __call__(self, *concat_inputs):
        out_arrs = self.sharded(*concat_inputs, *self.zeros_dev)
        return [
            np.asarray(out_arrs[i]).reshape(C, *self.zero_specs[i][0])
            for i in range(len(self.out_names))
        ]


def _get_runner(st, with_bias1, with_bias2):
    key = (
        st["tpc"],
        st["nseg"],
        st["lo_lim"],
        st["ncht"],
        tuple(st["KL"]),
        tuple(st["KH"]),
        with_bias1,
        with_bias2,
    )
    r = _runner_cache.get(key)
    if r is None:
        nc = build_program(st, with_bias1, with_bias2)
        r = _Runner(nc)
        _runner_cache[key] = r
    return r


def _fill_blob_weights(blob, st, W1, b1, W2, b2, wb1, wb2):
    offs, _ = _blob_fields(st, wb1, wb2)
    o, r, c = offs["w1"]
    blob[:, o : o + r * c] = (
        np.asarray(W1, np.float32).astype(ml_dtypes.bfloat16).view(np.int16).reshape(-1)
    )[None, :]
    o, r, c = offs["w2"]
    blob[:, o : o + r * c] = (
        np.asarray(W2, np.float32).astype(ml_dtypes.bfloat16).view(np.int16).reshape(-1)
    )[None, :]
    if wb1:
        o, r, c = offs["b1"]
        b1_t = np.tile(np.asarray(b1, np.float32), (P, 1))
        blob[:, o : o + r * c] = b1_t.view(np.int16).reshape(-1)[None, :]
    if wb2:
        o, r, c = offs["b2"]
        b2_t = np.tile(np.asarray(b2, np.float32), (P, 1))
        blob[:, o : o + r * c] = b2_t.view(np.int16).reshape(-1)[None, :]


def _pack_xs(x, n_nodes, npad):
    xs = np.zeros((npad, IN_CH), dtype=ml_dtypes.bfloat16)
    xs[:n_nodes] = x.astype(ml_dtypes.bfloat16)
    return xs


# ---------------------------------------------------------------- warmup

_warm_ready = False


def _warmup():
    global _warm_ready
    if os.environ.get("GCN_NO_PREBUILD"):
        return
    try:
        st = _uniform_st()
        runner = _get_runner(st, False, False)
        _, blob_w = _blob_fields(st, False, False)
        xs = np.zeros((C * TPC, IN_CH), ml_dtypes.bfloat16)
        blob = np.zeros((C * blob_w,), np.int16)
        runner(xs, blob)
        # warm the exact real-call path too (device-resident xs arg)
        xs_dev = runner.device_put_async(xs)
        runner(xs_dev, blob)
        _warm_ready = True
    except Exception:
        import traceback

        traceback.print_exc()
        _warm_ready = False


# ---------------------------------------------------------------- entry


def _run(x, edge_index, W1, b1, W2, b2, n_nodes, tpc, seg_tiles, lo_lim=LO_LIM,
         use_sim=False, uniform=False):
    wb1 = bool(np.any(np.asarray(b1) != 0))
    wb2 = bool(np.any(np.asarray(b2) != 0))
    extra = dict(wb1=wb1, wb2=wb2)

    runner = None
    xs_dev = None
    if uniform and not use_sim and not wb1 and not wb2:
        # start the (big) feature upload before edge preprocessing
        try:
            runner = _get_runner(_uniform_st(tpc, seg_tiles), False, False)
            xs_dev = runner.device_put_async(_pack_xs(x, n_nodes, C * tpc))
        except Exception:
            runner = None

    KL = KH = None
    if uniform and not wb1 and not wb2:
        KL, KH = KL_UNIFORM, KH_UNIFORM
    st, blob, fits = _preprocess(
        x, edge_index, n_nodes, tpc, seg_tiles, lo_lim, KL, KH, extra
    )
    if not fits:  # caps exceeded -> data-driven structure
        st, blob, _ = _preprocess(
            x, edge_index, n_nodes, tpc, seg_tiles, lo_lim, None, None, extra
        )
        runner = None
        xs_dev = None

    _fill_blob_weights(blob, st, W1, b1, W2, b2, wb1, wb2)

    if use_sim:
        from concourse import bass_interp

        key = ("sim", st["ncht"], tuple(st["KL"]), tuple(st["KH"]), wb1, wb2)
        nc = _prog_cache.get(key)
        if nc is None:
            nc = build_program(st, wb1, wb2)
            _prog_cache[key] = nc
        xs = _pack_xs(x, n_nodes, C * tpc).reshape(C, tpc, IN_CH)
        sim = bass_interp.MultiCoreSim(nc, C)
        for c in range(C):
            sim.cores[c].tensor("xs_t")[:] = xs[c]
            sim.cores[c].tensor("blob")[:] = blob[c]
        sim.simulate()
        outs = np.stack([sim.cores[c].mem_tensor("out") for c in range(C)])
    else:
        if runner is None:
            runner = _get_runner(st, wb1, wb2)
        if xs_dev is None:
            xs_dev = _pack_xs(x, n_nodes, C * tpc)
        outs = runner(xs_dev, blob.reshape(-1))[0]

    full = outs.reshape(C * tpc, OUT_CH)[:n_nodes]
    return np.asarray(full, dtype=np.float32)


def _gcn_host(x, edge_index, W1, b1, W2, b2):
    """Pure-numpy fallback (used only if the device path fails)."""
    n = x.shape[0]
    row = edge_index[0].astype(np.int64)
    col = edge_index[1].astype(np.int64)
    loops = np.arange(n, dtype=np.int64)
    row_f = np.concatenate([row, loops])
    col_f = np.concatenate([col, loops])
    deg = np.bincount(col_f, minlength=n).astype(np.float32)
    dinv = np.where(deg > 0, 1.0 / np.sqrt(deg), 0.0).astype(np.float32)
    norm = (dinv[row_f] * dinv[col_f]).astype(np.float32)
    order = np.argsort(col_f, kind="stable")
    row_sv = row_f[order]
    col_sv = col_f[order]
    norm_sv = norm[order][:, None]
    starts = np.searchsorted(col_sv, np.arange(n, dtype=np.int64))

    def conv(h, W, b):
        msg = norm_sv * (h @ W)[row_sv]
        return np.add.reduceat(msg, starts, axis=0) + b

    h = np.maximum(conv(x, W1, b1), 0.0)
    return conv(h, W2, b2).astype(np.float32)


def kernel(x, edge_index, W1, b1, W2, b2):
    x = np.asarray(x, dtype=np.float32)
    edge_index = np.asarray(edge_index)
    assert x.shape[0] == N_NODES
    args = (
        x,
        edge_index,
        np.asarray(W1, np.float32),
        np.asarray(b1, np.float32),
        np.asarray(W2, np.float32),
        np.asarray(b2, np.float32),
    )
    try:
        return _run(*args, N_NODES, tpc=TPC, seg_tiles=SEG_TILES, uniform=True)
    except Exception:
        import traceback

        traceback.print_exc()
        return _gcn_host(*args)


_warmup()
